# revision 3
# baseline (speedup 1.0000x reference)
"""Trainium2 Bass kernel for nn_CvxpyLayersSolver.

Computes, for every 2-D point p in a cloud of N=2,000,000 points, the
closed-form solution of the box-polytope dual LP:

    w   = p - clip(p, -1, 1)          (per coordinate)
    dist = ||w||
    mu  = [relu(w0), relu(-w0), relu(w1), relu(-w1)] / dist   (0 if dist==0)
    lam = [-w0, -w1, 0] / dist                                 (0 if dist==0)

Returns (mu.T, lam.T) with shapes (4, N) and (3, N), float32.

Sharding: pure data parallelism — each of the 8 NeuronCores processes a
contiguous slice of 250,000 points (padded to 250,112 = 128*1954 so the
per-core work maps exactly onto 128 SBUF partitions).
"""

import numpy as np

P = 128
N_CORES = 8
N_FULL = 2_000_000
PER_CORE = 250_000
F_TOTAL = 1954                 # points per partition per core
NP_CORE = P * F_TOTAL          # 250,112 padded points per core
CHUNK_FS = [490, 488, 488, 488]  # per-partition chunk widths; sum == F_TOTAL
assert sum(CHUNK_FS) == F_TOTAL

_NC_CACHE = {}


def _build_nc():
    from concourse import bacc, mybir
    from concourse.tile import TileContext

    f32 = mybir.dt.float32
    alu = mybir.AluOpType
    AF = mybir.ActivationFunctionType

    nc = bacc.Bacc(
        "TRN2", target_bir_lowering=False, debug=False, num_devices=N_CORES
    )

    # register the sqrt-guard bias as a const AP (same pattern as Bass ctor)
    _bias = 1e-38
    _bias_t = nc.alloc_sbuf_tensor("const-float32-bias", [128, 1], f32)
    nc.gpsimd.memset(_bias_t.ap(), _bias)
    nc.const_aps.aps[(f32, _bias)] = _bias_t.ap()
    nc.all_engine_barrier()

    pts = nc.dram_tensor("pts", [NP_CORE, 2], f32, kind="ExternalInput")
    out = nc.dram_tensor("out", [6, NP_CORE], f32, kind="ExternalOutput")

    ptsf = pts.ap().rearrange("n two -> (n two)")  # flat interleaved x,y
    outv = out.ap()

    with TileContext(nc) as tc:
        with (
            tc.tile_pool(name="io", bufs=3) as io,
            tc.tile_pool(name="wk", bufs=2) as wk,
        ):
            bc = 0  # chunk start, in points
            for F in CHUNK_FS:
                C = P * F
                in_view = ptsf[2 * bc : 2 * (bc + C)].rearrange(
                    "(p f) -> p f", p=P
                )  # (128, 2F): partition p holds F interleaved (x,y) pairs
                xy = io.tile([P, 2 * F], f32, tag="xy")
                nc.sync.dma_start(out=xy[:], in_=in_view)

                xyv = xy[:].rearrange("p (f two) -> p f two", two=2)
                x = xyv[:, :, 0]
                y = xyv[:, :, 1]

                # w = p - clip(p, -1, 1), per coordinate
                clipx = wk.tile([P, F], f32, tag="clipx")
                nc.vector.tensor_scalar(
                    clipx[:], x, 1.0, -1.0, alu.min, alu.max
                )
                w0 = wk.tile([P, F], f32, tag="w0")
                nc.vector.tensor_tensor(w0[:], x, clipx[:], alu.subtract)
                clipy = wk.tile([P, F], f32, tag="clipy")
                nc.vector.tensor_scalar(
                    clipy[:], y, 1.0, -1.0, alu.min, alu.max
                )
                w1 = wk.tile([P, F], f32, tag="w1")
                nc.vector.tensor_tensor(w1[:], y, clipy[:], alu.subtract)

                # dist = sqrt(w0^2 + w1^2 + tiny); tiny guards 1/0 (w==0 there)
                sq0 = wk.tile([P, F], f32, tag="sq0")
                nc.scalar.square(sq0[:], w0[:])
                sq1 = wk.tile([P, F], f32, tag="sq1")
                nc.scalar.square(sq1[:], w1[:])
                d2 = wk.tile([P, F], f32, tag="d2")
                nc.vector.tensor_add(d2[:], sq0[:], sq1[:])
                dist = wk.tile([P, F], f32, tag="dist")
                nc.scalar.activation(dist[:], d2[:], AF.Sqrt, bias=_bias)

                inv = wk.tile([P, F], f32, tag="inv")
                nc.vector.reciprocal_approx_fast(out=inv[:], in_=dist[:])

                t0 = wk.tile([P, F], f32, tag="t0")
                nc.vector.tensor_mul(t0[:], w0[:], inv[:])
                t1 = wk.tile([P, F], f32, tag="t1")
                nc.vector.tensor_mul(t1[:], w1[:], inv[:])

                # outputs: rows [mu0 mu1 mu2 mu3 lam0 lam1] of the chunk
                ot = io.tile([P, 6 * F], f32, tag="ot")
                nc.scalar.activation(ot[:, 0 * F : 1 * F], t0[:], AF.Relu)
                nc.scalar.activation(
                    ot[:, 1 * F : 2 * F], t0[:], AF.Relu, scale=-1.0
                )
                nc.scalar.activation(ot[:, 2 * F : 3 * F], t1[:], AF.Relu)
                nc.scalar.activation(
                    ot[:, 3 * F : 4 * F], t1[:], AF.Relu, scale=-1.0
                )
                nc.scalar.mul(ot[:, 4 * F : 5 * F], t0[:], -1.0)
                nc.scalar.mul(ot[:, 5 * F : 6 * F], t1[:], -1.0)

                out_view = outv[:, bc : bc + C].rearrange(
                    "r (p f) -> p r f", p=P
                )
                nc.sync.dma_start(
                    out=out_view,
                    in_=ot[:].rearrange("p (r f) -> p r f", r=6),
                )
                bc += C

    nc.compile()
    return nc


def _get_nc():
    if "nc" not in _NC_CACHE:
        _NC_CACHE["nc"] = _build_nc()
    return _NC_CACHE["nc"]


def _make_in_maps(pc):
    in_maps = []
    for c in range(N_CORES):
        buf = np.zeros((NP_CORE, 2), np.float32)
        buf[:PER_CORE] = pc[c * PER_CORE : (c + 1) * PER_CORE]
        in_maps.append({"pts": buf})
    return in_maps


def _gather(results):
    mu = np.empty((4, N_FULL), np.float32)
    lam = np.empty((3, N_FULL), np.float32)
    lam[2] = 0.0
    for c in range(N_CORES):
        o = results[c]["out"]
        sl = slice(c * PER_CORE, (c + 1) * PER_CORE)
        mu[:, sl] = o[0:4, :PER_CORE]
        lam[0:2, sl] = o[4:6, :PER_CORE]
    return mu, lam


def run_on_hw(pc, trace=False, **kwargs):
    from concourse.bass_utils import run_bass_kernel_spmd

    nc = _get_nc()
    in_maps = _make_in_maps(pc)
    res = run_bass_kernel_spmd(
        nc, in_maps, list(range(N_CORES)), trace=trace, **kwargs
    )
    return _gather(res.results), res


def kernel(point_cloud, G=None, h=None):
    pc = np.ascontiguousarray(np.asarray(point_cloud, dtype=np.float32))
    (mu, lam), _ = run_on_hw(pc)
    return mu, lam
